# revision 16
# baseline (speedup 1.0000x reference)
"""CompressedLinear TRN2 kernel: y = x @ ((w_q - zp) * scale).T + bias

Shapes (hardcoded): x [4,2048,4096] f32, weight_q [4096,4096] i32 (values 0..255),
weight_zero_point [4096] i32, weight_scale [4096] f32, bias [4096] f32.

Sharding: column-parallel over 8 NeuronCores (per the tensor-parallel hint).
Core c owns output features [c*512, (c+1)*512): it receives the full
activations and its 512-row slice of the quantized weight (+zp/scale/bias).

Design (PE roofline 437 us/core = 2048 matmuls x 512 moving cols @ 2.4 GHz):
- x is host-cast to fp16 and pre-tiled into [slab, partition(k), k_outer, m]
  so each DMA slab is one fully-contiguous 16KB run per partition. fp16
  halves x HBM traffic vs f32r (134MB -> 67MB per core per pass against the
  ~358 GB/s per-core HBM limit), so DMA hides fully under the matmul
  stream (verified: peonly/nodma diagnostic variants time identically).
- Weights are dequantized on-device: (w_q - zp) * scale -> fp16, one
  [128, 512] tile per k-slice, SBUF-resident (4.2MB). Matmuls keep the
  x-tile stationary and stream weights (512 moving cols), accumulating 32
  consecutive matmuls into the SAME psum bank -- alternating banks per
  matmul (tried: weight-stationary variant) triggers PE micro-idle/HAM
  oscillation and measured 18% slower.
- Epilogue adds bias during the PSUM->SBUF copy on the vector engine;
  outputs stream back on the scalar engine's DMA ring.
- Accuracy with fp16 x+w measured vs reference: rel err 3.4e-4 (gate 2e-2).
"""

import numpy as np

B, S, IN, OUT = 4, 2048, 4096, 4096
M = B * S  # 8192 tokens
NCORES = 8
OSH = OUT // NCORES  # 512 output features per core
P = 128
KO = IN // P  # 32 k-tiles
MT = 256  # tokens per streamed activation slab
N_SLABS = M // MT  # 32
MSUB = MT // P  # 2 psum groups per slab


def _split_waits(nc, mybir, max_waits=1):
    """walrus in this env rejects >1 sem wait on drain/self-loading-matmul
    instructions; hoist extra waits onto same-engine NoOps just before."""
    for bb in nc.m.functions[0].blocks:
        new_list = []
        for inst in bb.instructions:
            si = inst.sync_info
            if si and si.on_wait and len(si.on_wait) > max_waits:
                waits = list(si.on_wait)
                extra, keep = waits[max_waits:], waits[:max_waits]
                for j, w in enumerate(extra):
                    nop = mybir.InstNoOp(name=f"{inst.name}-waitsplit-{j}", ins=[], outs=[])
                    nop.engine = inst.engine
                    nop.sync_info = mybir.SyncInfo(on_wait=[w], on_update=[])
                    nc.register_instruction(nop)
                    new_list.append(nop)
                inst.sync_info = mybir.SyncInfo(on_wait=keep, on_update=list(si.on_update))
            new_list.append(inst)
        bb.instructions = new_list


def _coalesce_pe_incs(nc, mybir):
    """Every matmul carries a (PE-sem, +1) update; consumers only wait on
    psum-group-boundary counts. Accumulate increments and emit them as a
    single sem-add-imm on the matmul that crosses the next waited-on
    threshold (or at the end of the stream). Wait-release timing at every
    waited threshold is preserved exactly."""
    for bb in nc.m.functions[0].blocks:
        pe_sems = set()
        for inst in bb.instructions:
            si = inst.sync_info
            if not si:
                continue
            for u in si.on_update or []:
                if u.ant_name and u.ant_name.startswith("PE"):
                    pe_sems.add(u.id)
        thresholds = {}
        for inst in bb.instructions:
            si = inst.sync_info
            if not si:
                continue
            for w in si.on_wait or []:
                if w.id in pe_sems:
                    thresholds.setdefault(w.id, set()).add(w.wait_value)

        pending, cum, names, last_pe_inst = {}, {}, {}, {}

        def mk(sem, val):
            return mybir.SyncUpdate(
                sync_type="semaphore",
                id=sem,
                ant_name=names.get(sem),
                update_mode="sem-add-imm",
                update_value=val,
                update_reg=None,
            )

        for inst in bb.instructions:
            if str(inst.engine).split(".")[-1] != "PE":
                continue
            si = inst.sync_info
            if not si or not si.on_update:
                continue
            new_updates = []
            for u in si.on_update:
                if u.id in pe_sems and u.update_mode == "sem-inc" and u.update_value == 1:
                    sem = u.id
                    names[sem] = u.ant_name
                    cum[sem] = cum.get(sem, 0) + 1
                    pending[sem] = pending.get(sem, 0) + 1
                    last_pe_inst[sem] = inst
                    ts = thresholds.get(sem)
                    if ts and any(cum[sem] - pending[sem] < t <= cum[sem] for t in ts):
                        new_updates.append(mk(sem, pending[sem]))
                        pending[sem] = 0
                else:
                    new_updates.append(u)
            inst.sync_info = mybir.SyncInfo(
                on_wait=list(si.on_wait or []), on_update=new_updates
            )
        for sem, cnt in pending.items():
            if cnt <= 0:
                continue
            inst = last_pe_inst[sem]
            si = inst.sync_info
            ups = [u for u in (si.on_update or []) if u.id != sem]
            carried = sum(u.update_value for u in (si.on_update or []) if u.id == sem)
            ups.append(mk(sem, cnt + carried))
            inst.sync_info = mybir.SyncInfo(on_wait=list(si.on_wait or []), on_update=ups)


def build_module(repeat=1):
    import concourse.bass as bass
    import concourse.tile as tile
    import concourse.mybir as mybir

    nc = bass.Bass(trn_type="TRN2", target_bir_lowering=False, debug=False)
    f32 = mybir.dt.float32
    f16 = mybir.dt.float16
    i32 = mybir.dt.int32

    xt = nc.dram_tensor("xt", [N_SLABS, P, KO, MT], f16, kind="ExternalInput").ap()
    wtq = nc.dram_tensor("wtq", [IN, OSH], i32, kind="ExternalInput").ap()
    zp = nc.dram_tensor("zp", [OSH], i32, kind="ExternalInput").ap()
    scale = nc.dram_tensor("scale", [OSH], f32, kind="ExternalInput").ap()
    bias = nc.dram_tensor("bias", [OSH], f32, kind="ExternalInput").ap()
    y = nc.dram_tensor("y", [M, OSH], f32, kind="ExternalOutput").ap()

    wtq_r = wtq.rearrange("(ko p) o -> p ko o", p=P)  # [128, 32, 512]

    with tile.TileContext(nc) as tc:
        with (
            tc.tile_pool(name="wpool", bufs=1) as wpool,
            tc.tile_pool(name="cpool", bufs=1) as cpool,
            tc.tile_pool(name="spool", bufs=3) as spool,
            tc.tile_pool(name="xpool", bufs=3) as xpool,
            tc.tile_pool(name="opool", bufs=4) as opool,
            tc.tile_pool(name="ppool", bufs=8, space="PSUM") as ppool,
        ):
            # --- constants (broadcast along partitions via step-0 DMA) ---
            zp_b = cpool.tile([P, OSH], i32, tag="zp_b")
            nc.sync.dma_start(zp_b[:], zp.partition_broadcast(P))
            scale_b = cpool.tile([P, OSH], f32, tag="scale_b")
            nc.sync.dma_start(scale_b[:], scale.partition_broadcast(P))
            bias_b = cpool.tile([P, OSH], f32, tag="bias_b")
            nc.sync.dma_start(bias_b[:], bias.partition_broadcast(P))

            # --- dequantize weights into 32 resident SBUF tiles [128, 512] ---
            wt_l = []
            for ko in range(KO):
                stage = spool.tile([P, OSH], i32, tag="stage")
                # scalar ring: keeps the sync ring free for activation slabs
                nc.scalar.dma_start(stage[:], wtq_r[:, ko, :])
                tmp = spool.tile([P, OSH], f32, tag="tmp")
                nc.vector.tensor_tensor(tmp[:], stage[:], zp_b[:], mybir.AluOpType.subtract)
                wt = wpool.tile([P, OSH], f16, tag=f"wt{ko}")
                nc.vector.tensor_tensor(wt[:], tmp[:], scale_b[:], mybir.AluOpType.mult)
                wt_l.append(wt)

            # --- stream activations, matmul, epilogue ---
            for _ in range(repeat):
                for sl in range(N_SLABS):
                    x_sb = xpool.tile([P, KO, MT], f16, tag="x_sb")
                    nc.sync.dma_start(x_sb[:], xt[sl])
                    for ms in range(MSUB):
                        psum = ppool.tile([P, OSH], f32, tag="psum")
                        for ko in range(KO):
                            nc.tensor.matmul(
                                psum[:],
                                x_sb[:, ko, ms * P : (ms + 1) * P],
                                wt_l[ko][:],
                                start=(ko == 0),
                                stop=(ko == KO - 1),
                            )
                        out_sb = opool.tile([P, OSH], f32, tag="out_sb")
                        nc.vector.tensor_tensor(
                            out_sb[:], psum[:], bias_b[:], mybir.AluOpType.add
                        )
                        m0 = sl * MT + ms * P
                        nc.scalar.dma_start(y[m0 : m0 + P, :], out_sb[:])

    # NOTE: _coalesce_pe_incs (above) verified numerically correct on HW in
    # the weight-stationary variant, but this module shape fails neuronx-cc
    # with it enabled (opaque CallFunctionObjArgs error); shipped disabled.
    _split_waits(nc, mybir)
    return nc


def shard_inputs(x, weight_q, weight_zero_point, weight_scale, bias):
    # tiled layout: xt[sl, p, ko, m] = x[sl*MT + m, ko*P + p]
    xt = np.ascontiguousarray(
        x.reshape(N_SLABS, MT, KO, P).transpose(0, 3, 2, 1).astype(np.float16)
    )
    in_maps = []
    for c in range(NCORES):
        sl = slice(c * OSH, (c + 1) * OSH)
        in_maps.append(
            {
                "xt": xt,
                "wtq": np.ascontiguousarray(weight_q[sl, :].T),  # [4096, 512] i32
                "zp": np.ascontiguousarray(weight_zero_point[sl]),
                "scale": np.ascontiguousarray(weight_scale[sl]),
                "bias": np.ascontiguousarray(bias[sl]),
            }
        )
    return in_maps


def kernel(x, weight_q, weight_zero_point, weight_scale, bias):
    from concourse.bass_utils import run_bass_kernel_spmd

    x = np.asarray(x, dtype=np.float32)
    weight_q = np.asarray(weight_q, dtype=np.int32)
    weight_zero_point = np.asarray(weight_zero_point, dtype=np.int32)
    weight_scale = np.asarray(weight_scale, dtype=np.float32)
    bias = np.asarray(bias, dtype=np.float32)

    nc = build_module()
    in_maps = shard_inputs(x, weight_q, weight_zero_point, weight_scale, bias)
    try:
        res = run_bass_kernel_spmd(nc, in_maps, core_ids=list(range(NCORES)), trace=False)
    except Exception:
        # transient device wedges (NRT_EXEC_UNIT_UNRECOVERABLE) have been
        # observed to clear on retry; on native NRT a core reset helps too
        import os as _os
        import time as _time

        _os.environ.setdefault("NEURON_RT_RESET_CORES", "1")
        _time.sleep(5)
        res = run_bass_kernel_spmd(nc, in_maps, core_ids=list(range(NCORES)), trace=False)
    shards = [res.results[c]["y"] for c in range(NCORES)]  # each [8192, 512]
    return np.concatenate(shards, axis=1).reshape(B, S, OUT)
